# revision 8
# baseline (speedup 1.0000x reference)
"""PoseODERNN Trainium2 kernel.

Data-parallel over batch: 8 cores x 32 batch elements. Per core the ODE-RNN
scan runs with batch-major activations [rows=64, feat], activation-stationary
matmuls in bf16 (fp32 PSUM accumulate, fp32 state), transposes on the PE via
identity matmuls, biases applied as K=1 bias-row matmuls.
"""
import numpy as np

try:
    import concourse.bass as bass
except ImportError:  # grading env may not have the repo on sys.path
    import sys

    sys.path.insert(0, "/opt/trn_rl_repo")
    import concourse.bass as bass

import concourse.bacc as bacc
import concourse.tile as tile
import concourse.mybir as mybir
import ml_dtypes
from concourse.bass_utils import run_bass_kernel_spmd

BF = mybir.dt.bfloat16
F32 = mybir.dt.float32
AF = mybir.ActivationFunctionType

B, S, VF, IF = 256, 32, 512, 256
F, H, G = 768, 1024, 2304
L, KE = 2, 8
NCORES = 8
BL = B // NCORES  # 32 batch per core
R = L * BL  # 64 state rows per core
FK, HK = F // 128, H // 128  # 6, 8 contraction tiles


def _nchunks(n):
    """Split n columns into <=512 chunks."""
    out = []
    c = 0
    while c < n:
        w = min(512, n - c)
        out.append((c, w))
        c += w
    return out


def build(n_steps=S):
    nc = bacc.Bacc(
        "TRN2", target_bir_lowering=False, debug=False, num_devices=NCORES
    )

    def din(name, shape, dt=BF):
        return nc.dram_tensor(name, shape, dt, kind="ExternalInput").ap()

    xT_d = din("xT", [F, n_steps * BL])
    w0_d = din("w0T", [F, H])
    w1_d = din("w1T", [H, H])
    w2_d = din("w2T", [H, F])
    wx1_d = din("wx1T", [F, G])
    wh1_d = din("wh1T", [F, G])
    wx2_d = din("wx2T", [F, G])
    wh2_d = din("wh2T", [F, G])
    rw1_d = din("rw1T", [F, 128])
    rw2_d = din("rw2T", [128, 8])  # 6 cols padded to 8
    b0_d = din("b0r", [1, H])
    b1_d = din("b1r", [1, H])
    b2_d = din("b2r", [1, F])
    bs1_d = din("bs1r", [1, G])  # bx0 + [bh0_r, bh0_z, 0]
    bs2_d = din("bs2r", [1, G])
    bhn1_d = din("bhn1r", [1, F])  # bh0_n
    bhn2_d = din("bhn2r", [1, F])
    rb1_d = din("rb1r", [1, 128])
    rb2_d = din("rb2r", [1, 8])
    ident_d = din("ident", [64, 64])
    subs_d = din("subs", [R, n_steps], F32)

    pose_d = nc.dram_tensor(
        "pose", [BL, n_steps * 8], F32, kind="ExternalOutput"
    ).ap()
    hlast_d = nc.dram_tensor("hlast", [R, F], F32, kind="ExternalOutput").ap()

    with tile.TileContext(nc) as tc:
        _body(nc, tc, n_steps, locals())
    nc.compile()
    return nc


def _body(nc, tc, n_steps, d):
    from contextlib import ExitStack

    ctx = ExitStack()
    with ctx:
        wp = ctx.enter_context(tc.tile_pool(name="weights", bufs=1))
        st = ctx.enter_context(tc.tile_pool(name="state", bufs=1))
        dram = ctx.enter_context(tc.tile_pool(name="dram", bufs=1, space="DRAM"))

        # ---- persistent weights ----
        def wload(ap_d, kt, n):
            t = wp.tile([128, kt, n], BF, tag=ap_d.tensor.name)
            for k in range(kt):
                nc.sync.dma_start(t[:, k, :], ap_d[k * 128 : (k + 1) * 128, :])
            return t

        w0 = wload(d["w0_d"], FK, H)
        w1 = wload(d["w1_d"], HK, H)
        w2 = wload(d["w2_d"], HK, F)
        wh1 = wload(d["wh1_d"], FK, G)
        wx2 = wload(d["wx2_d"], FK, G)
        wh2 = wload(d["wh2_d"], FK, G)
        rw1 = wload(d["rw1_d"], FK, 128)
        rw2 = wp.tile([128, 8], BF)
        nc.sync.dma_start(rw2[:, :], d["rw2_d"][:, :])

        def brow(ap_d, n):
            t = wp.tile([1, n], BF, tag=ap_d.tensor.name)
            nc.sync.dma_start(t[:, :], ap_d[:, :])
            return t

        b0r = brow(d["b0_d"], H)
        b1r = brow(d["b1_d"], H)
        b2r = brow(d["b2_d"], F)
        bs2r = brow(d["bs2_d"], G)
        bhn1r = brow(d["bhn1_d"], F)
        bhn2r = brow(d["bhn2_d"], F)
        rb1r = brow(d["rb1_d"], 128)
        rb2r = brow(d["rb2_d"], 8)

        ident = wp.tile([64, 64], BF)
        nc.sync.dma_start(ident[:, :], d["ident_d"][:, :])
        ones = wp.tile([1, 128], BF)
        nc.vector.memset(ones[:, :], 1.0)
        subs = st.tile([R, n_steps], F32)
        nc.sync.dma_start(subs[:, :], d["subs_d"][:, :])

        # ---- state ----
        y32 = st.tile([R, F], F32)  # authoritative state [L*BL, F]
        nc.vector.memset(y32[:, :], 0.0)
        pose_sb = st.tile([R, n_steps * 8], F32)
        gx1_dram = dram.tile([n_steps * BL, G], BF)

        # ---- precompute GX1 = x @ Wx1.T + bs1 for all steps ----
        with (
            tc.tile_pool(name="pre", bufs=1) as pre,
            tc.tile_pool(name="pre2", bufs=2) as pre2,
            tc.tile_pool(name="pre_ps", bufs=2, space="PSUM") as pre_ps,
        ):
            nrows = n_steps * BL
            xT = pre.tile([128, FK, nrows], BF)
            for k in range(FK):
                nc.sync.dma_start(xT[:, k, :], d["xT_d"][k * 128 : (k + 1) * 128, :])
            wx1 = pre.tile([128, FK, G], BF)
            for k in range(FK):
                nc.sync.dma_start(
                    wx1[:, k, :], d["wx1_d"][k * 128 : (k + 1) * 128, :]
                )
            bs1r = pre.tile([1, G], BF)
            nc.sync.dma_start(bs1r[:, :], d["bs1_d"][:, :])
            for m in range(0, nrows, 128):
                mw = min(128, nrows - m)
                stage = pre2.tile([128, G], BF, tag="stage")
                for (c, wdt) in _nchunks(G):
                    ps = pre_ps.tile([128, 512], F32, tag="ps")
                    for k in range(FK):
                        nc.tensor.matmul(
                            ps[:mw, :wdt],
                            xT[:, k, m : m + mw],
                            wx1[:, k, c : c + wdt],
                            start=(k == 0),
                            stop=False,
                        )
                    nc.tensor.matmul(
                        ps[:mw, :wdt],
                        ones[:, :mw],
                        bs1r[:, c : c + wdt],
                        start=False,
                        stop=True,
                    )
                    nc.scalar.copy(stage[:mw, c : c + wdt], ps[:mw, :wdt])
                nc.sync.dma_start(gx1_dram[m : m + mw, :], stage[:mw, :])

        # ---- the scan ----
        ctx2 = ExitStack()
        with ctx2:
            ap = ctx2.enter_context(tc.tile_pool(name="act", bufs=2))
            sp = ctx2.enter_context(tc.tile_pool(name="small", bufs=2))
            psp = ctx2.enter_context(tc.tile_pool(name="ps", bufs=3, space="PSUM"))
            trp = psp

            def transpose_rows(src_bf, rows, kt, tag, p0=0):
                # src_bf rows [p0:p0+rows, kt*128] bf16 -> dst [128, kt, rows]
                pst = trp.tile([128, kt * rows], BF, tag="tr")
                for k in range(kt):
                    nc.tensor.transpose(
                        pst[:, k * rows : (k + 1) * rows],
                        src_bf[p0 : p0 + rows, k * 128 : (k + 1) * 128],
                        ident[p0 : p0 + rows, p0 : p0 + rows],
                    )
                dst = ap.tile([128, kt, rows], BF, tag=tag)
                nc.scalar.copy(
                    dst[:, :, :],
                    pst[:, : kt * rows].rearrange("p (k r) -> p k r", k=kt),
                )
                return dst

            def mlp_layer(xT_t, kt, wT, brow_t, nout, outf, act, rows=R, p0=0):
                # outf[p0:p0+rows, nout] <- act(xT_t.T @ wT + brow)
                tp = (0, p0) if p0 else None
                for (c, wdt) in _nchunks(nout):
                    ps = psp.tile([64, 512], F32, tag="mm")
                    for k in range(kt):
                        nc.tensor.matmul(
                            ps[p0 : p0 + rows, :wdt],
                            xT_t[:, k, :rows],
                            wT[:, k, c : c + wdt],
                            start=(k == 0),
                            stop=False,
                            tile_position=tp,
                        )
                    nc.tensor.matmul(
                        ps[p0 : p0 + rows, :wdt],
                        ones[:, :rows],
                        brow_t[:, c : c + wdt],
                        start=False,
                        stop=True,
                        tile_position=tp,
                    )
                    nc.scalar.activation(
                        outf[p0 : p0 + rows, c : c + wdt],
                        ps[p0 : p0 + rows, :wdt],
                        act,
                    )

            def gru_layer(p0, gx_sb, hT, wh, bhn, h_rows):
                # all tiles band-sliced at [p0:p0+BL]; gx_sb pre-sliced AP
                tp = (0, p0) if p0 else None
                pe = p0 + BL
                ghn = sp.tile([R, F], BF, tag="ghn")
                rz = sp.tile([R, 2 * F], BF, tag="rz")
                for ci, (c, wdt) in enumerate(_nchunks(G)):
                    ps = psp.tile([64, 512], F32, tag="mm")
                    for k in range(FK):
                        nc.tensor.matmul(
                            ps[p0:pe, :wdt],
                            hT[:, k, :],
                            wh[:, k, c : c + wdt],
                            start=(k == 0),
                            stop=(k == FK - 1 and c + wdt <= 2 * F),
                            tile_position=tp,
                        )
                    if c + wdt > 2 * F:  # n-section: add bh_n
                        nc.tensor.matmul(
                            ps[p0:pe, :wdt],
                            ones[:, :BL],
                            bhn[:, c - 2 * F : c - 2 * F + wdt],
                            start=False,
                            stop=True,
                            tile_position=tp,
                        )
                    if c + wdt <= 2 * F:
                        # r/z section: rz_pre = gx + gh
                        nc.vector.tensor_add(
                            rz[p0:pe, c : c + wdt],
                            gx_sb[:, c : c + wdt],
                            ps[p0:pe, :wdt],
                        )
                    else:
                        nc.scalar.copy(
                            ghn[p0:pe, c - 2 * F : c - 2 * F + wdt],
                            ps[p0:pe, :wdt],
                        )
                nc.scalar.activation(rz[p0:pe, :], rz[p0:pe, :], AF.Sigmoid)
                # n = tanh(gx_n + r*ghn)
                npre = sp.tile([R, F], BF, tag="npre")
                nc.vector.tensor_mul(npre[p0:pe, :], rz[p0:pe, :F], ghn[p0:pe, :])
                nc.vector.tensor_add(npre[p0:pe, :], npre[p0:pe, :], gx_sb[:, 2 * F :])
                nc.scalar.activation(npre[p0:pe, :], npre[p0:pe, :], AF.Tanh)
                # h' = n + z*(h - n)
                hmn = sp.tile([R, F], BF, tag="hmn")
                nc.vector.tensor_sub(hmn[p0:pe, :], h_rows, npre[p0:pe, :])
                nc.vector.tensor_mul(hmn[p0:pe, :], rz[p0:pe, F : 2 * F], hmn[p0:pe, :])
                nc.vector.tensor_add(h_rows, npre[p0:pe, :], hmn[p0:pe, :])

            for s in range(n_steps):
                gx1_sb = ap.tile([BL, G], BF, tag="gx1")
                nc.sync.dma_start(
                    gx1_sb[:, :], gx1_dram[s * BL : (s + 1) * BL, :]
                )
                # ---- KE Euler substeps ----
                for e in range(KE):
                    ybf = ap.tile([R, F], BF, tag="ybf")
                    nc.vector.tensor_copy(ybf[:, :], y32[:, :])
                    yT = transpose_rows(ybf, R, FK, "yT")
                    h1 = ap.tile([R, H], BF, tag="h1")
                    mlp_layer(yT, FK, w0, b0r, H, h1, AF.Tanh)
                    h1T = transpose_rows(h1, R, HK, "h1T")
                    h2 = ap.tile([R, H], BF, tag="h2")
                    mlp_layer(h1T, HK, w1, b1r, H, h2, AF.Tanh)
                    h2T = transpose_rows(h2, R, HK, "h2T")
                    # f = h2 @ W2.T + b2; y += sub*f
                    tmp = sp.tile([R, F], F32, tag="tmp")
                    for (c, wdt) in _nchunks(F):
                        ps = psp.tile([64, 512], F32, tag="mm")
                        for k in range(HK):
                            nc.tensor.matmul(
                                ps[:, :wdt],
                                h2T[:, k, :],
                                w2[:, k, c : c + wdt],
                                start=(k == 0),
                                stop=False,
                            )
                        nc.tensor.matmul(
                            ps[:, :wdt],
                            ones[:, :R],
                            b2r[:, c : c + wdt],
                            start=False,
                            stop=True,
                        )
                        nc.scalar.activation(
                            tmp[:, c : c + wdt],
                            ps[:, :wdt],
                            AF.Copy,
                            scale=subs[:, s : s + 1],
                        )
                    nc.vector.tensor_add(y32[:, :], y32[:, :], tmp[:, :])

                # ---- GRU ----
                ybf = ap.tile([R, F], BF, tag="ybf")
                nc.vector.tensor_copy(ybf[:, :], y32[:, :])
                yT = transpose_rows(ybf, R, FK, "yT")
                gru_layer(0, gx1_sb[:, :], yT[:, :, :BL], wh1, bhn1r, y32[:BL, :])
                # gx2 = h1out @ Wx2.T + bs2  (into band 32:64)
                h1o_bf = ap.tile([R, F], BF, tag="ybf")
                nc.vector.tensor_copy(h1o_bf[:BL, :], y32[:BL, :])
                h1oT = transpose_rows(h1o_bf, BL, FK, "yT")
                gx2_sb = ap.tile([R, G], BF, tag="gx2")
                mlp_layer(h1oT, FK, wx2, bs2r, G, gx2_sb, AF.Copy, rows=BL, p0=BL)
                gru_layer(BL, gx2_sb[BL:, :], yT[:, :, BL:], wh2, bhn2r, y32[BL:, :])

                # ---- regressor on h2out = y32[BL:] (band 32:64) ----
                h2o_bf = ap.tile([R, F], BF, tag="ybf")
                nc.vector.tensor_copy(h2o_bf[BL:, :], y32[BL:, :])
                h2oT = transpose_rows(h2o_bf, BL, FK, "yT", p0=BL)
                psr = psp.tile([64, 128], F32, tag="mm")
                for k in range(FK):
                    nc.tensor.matmul(
                        psr[BL:, :],
                        h2oT[:, k, :],
                        rw1[:, k, :],
                        start=(k == 0),
                        stop=False,
                        tile_position=(0, BL),
                    )
                nc.tensor.matmul(
                    psr[BL:, :],
                    ones[:, :BL],
                    rb1r[:, :],
                    start=False,
                    stop=True,
                    tile_position=(0, BL),
                )
                hid_bf = sp.tile([R, 128], BF, tag="hid")
                hid01 = sp.tile([R, 128], F32, tag="hid01")
                nc.vector.tensor_scalar_mul(hid01[BL:, :], psr[BL:, :], 0.1)
                nc.vector.tensor_max(hid_bf[BL:, :], psr[BL:, :], hid01[BL:, :])
                pst = trp.tile([128, 32], BF, tag="tr")
                nc.tensor.transpose(
                    pst[:, :], hid_bf[BL:, :], ident[BL:, BL:]
                )
                hidT = sp.tile([128, 32], BF, tag="hidT")
                nc.scalar.copy(hidT[:, :], pst[:, :])
                psr2 = psp.tile([64, 8], F32, tag="mm")
                nc.tensor.matmul(
                    psr2[BL:, :], hidT[:, :], rw2[:, :],
                    start=True, stop=False, tile_position=(0, BL),
                )
                nc.tensor.matmul(
                    psr2[BL:, :], ones[:, :BL], rb2r[:, :],
                    start=False, stop=True, tile_position=(0, BL),
                )
                nc.vector.tensor_copy(
                    pose_sb[BL:, s * 8 : (s + 1) * 8], psr2[BL:, :]
                )

            nc.sync.dma_start(d["pose_d"][:, :], pose_sb[BL:, :])
            nc.sync.dma_start(d["hlast_d"][:, :], y32[:, :])


# ---------------- host side ----------------


def _prep_core_inputs(inputs, core, n_steps=S):
    bf16 = ml_dtypes.bfloat16
    sl = slice(core * BL, (core + 1) * BL)
    fv, fi, ts = inputs["fv"][sl], inputs["fi"][sl], inputs["ts"][sl]
    x = np.concatenate([fv, fi], -1).astype(np.float32)  # [BL, S, F]
    x = np.swapaxes(x, 0, 1)[:n_steps]  # [S, BL, F]
    xT = np.ascontiguousarray(x.reshape(n_steps * BL, F).T).astype(bf16)
    dts = np.concatenate(
        [ts[:, 1:] - ts[:, :-1], np.zeros((BL, 1), np.float32)], 1
    )
    sub = (dts / KE).astype(np.float32)[:, :n_steps]  # [BL, n_steps]
    subs = np.concatenate([sub, sub], 0)  # [R, n_steps]

    bx, bh = inputs["gru_bx"], inputs["gru_bh"]

    def bsum(l):
        return np.concatenate(
            [bx[l][: 2 * F] + bh[l][: 2 * F], bx[l][2 * F :]]
        ).astype(np.float32)

    t = lambda a: np.ascontiguousarray(np.asarray(a, np.float32).T).astype(bf16)
    row = lambda v: np.asarray(v, np.float32).reshape(1, -1).astype(bf16)
    rw2 = np.zeros((128, 8), np.float32)
    rw2[:, :6] = inputs["reg_W2"].T
    rb2 = np.zeros((1, 8), np.float32)
    rb2[0, :6] = inputs["reg_b2"]
    return {
        "xT": xT,
        "w0T": t(inputs["ode_W0"]),
        "w1T": t(inputs["ode_W1"]),
        "w2T": t(inputs["ode_W2"]),
        "wx1T": t(inputs["gru_Wx"][0]),
        "wh1T": t(inputs["gru_Wh"][0]),
        "wx2T": t(inputs["gru_Wx"][1]),
        "wh2T": t(inputs["gru_Wh"][1]),
        "rw1T": t(inputs["reg_W1"]),
        "rw2T": rw2.astype(bf16),
        "b0r": row(inputs["ode_b0"]),
        "b1r": row(inputs["ode_b1"]),
        "b2r": row(inputs["ode_b2"]),
        "bs1r": row(bsum(0)),
        "bs2r": row(bsum(1)),
        "bhn1r": row(bh[0][2 * F :]),
        "bhn2r": row(bh[1][2 * F :]),
        "rb1r": row(inputs["reg_b1"]),
        "rb2r": rb2.astype(bf16),
        "ident": np.eye(64, dtype=bf16),
        "subs": subs,
    }


def run(inputs, n_steps=S, trace=False, nc=None, repeats=1, timings=None):
    inputs = {k: np.asarray(v) for k, v in inputs.items()}
    if nc is None:
        nc = build(n_steps)
    in_maps = [_prep_core_inputs(inputs, c, n_steps) for c in range(NCORES)]
    res = run_bass_kernel_spmd(
        nc, in_maps, core_ids=list(range(NCORES)), trace=trace
    )
    if repeats > 1 and timings is not None:
        import time as _time
        for _ in range(repeats - 1):
            t0 = _time.time()
            run_bass_kernel_spmd(
                nc, in_maps, core_ids=list(range(NCORES)), trace=False
            )
            timings.append(_time.time() - t0)
    pose = np.zeros((B, n_steps, 6), np.float32)
    hlast = np.zeros((L, B, F), np.float32)
    for c in range(NCORES):
        sl = slice(c * BL, (c + 1) * BL)
        pc = res.results[c]["pose"].reshape(BL, n_steps, 8)
        pose[sl] = pc[:, :, :6]
        hlast[:, sl] = res.results[c]["hlast"].reshape(L, BL, F)
    return (pose, hlast), res


def kernel(**inputs):
    (pose, hlast), _ = run(inputs)
    return pose, hlast


# revision 16
# speedup vs baseline: 1.1952x; 1.1952x over previous
"""PoseODERNN Trainium2 kernel.

Data-parallel over batch: 8 cores x 32 batch elements. Per core the ODE-RNN
scan runs with batch-major activations [rows=64, feat], activation-stationary
matmuls in bf16 (fp32 PSUM accumulate, fp32 state), transposes on the PE via
identity matmuls, biases applied as K=1 bias-row matmuls.
"""
import numpy as np

try:
    import concourse.bass as bass
except ImportError:  # grading env may not have the repo on sys.path
    import sys

    sys.path.insert(0, "/opt/trn_rl_repo")
    import concourse.bass as bass

import concourse.bacc as bacc
import concourse.tile as tile
import concourse.mybir as mybir
import ml_dtypes
from concourse.bass_utils import run_bass_kernel_spmd

BF = mybir.dt.bfloat16
F32 = mybir.dt.float32
AF = mybir.ActivationFunctionType

B, S, VF, IF = 256, 32, 512, 256
F, H, G = 768, 1024, 2304
L, KE = 2, 8
NCORES = 8
BL = B // NCORES  # 32 batch per core
R = L * BL  # 64 state rows per core
FK, HK = F // 128, H // 128  # 6, 8 contraction tiles


def _nchunks(n):
    """Split n columns into <=512 chunks."""
    out = []
    c = 0
    while c < n:
        w = min(512, n - c)
        out.append((c, w))
        c += w
    return out


def build(n_steps=S):
    nc = bacc.Bacc(
        "TRN2", target_bir_lowering=False, debug=False, num_devices=NCORES
    )

    def din(name, shape, dt=BF):
        return nc.dram_tensor(name, shape, dt, kind="ExternalInput").ap()

    xT_d = din("xT", [F, n_steps * BL])
    w0_d = din("w0T", [F, H])
    w1_d = din("w1T", [H, H])
    w2_d = din("w2T", [H, F])
    wx1_d = din("wx1T", [F, G])
    wh1_d = din("wh1T", [F, G])
    wx2_d = din("wx2T", [F, G])
    wh2_d = din("wh2T", [F, G])
    rw1_d = din("rw1T", [F, 128])
    rw2_d = din("rw2T", [128, 8])  # 6 cols padded to 8
    b0_d = din("b0r", [1, H])
    b1_d = din("b1r", [1, H])
    b2_d = din("b2r", [1, F])
    bs1_d = din("bs1r", [1, G])  # bx0 + [bh0_r, bh0_z, 0]
    bs2_d = din("bs2r", [1, G])
    bhn1_d = din("bhn1r", [1, F])  # bh0_n
    bhn2_d = din("bhn2r", [1, F])
    rb1_d = din("rb1r", [1, 128])
    rb2_d = din("rb2r", [1, 8])
    ident_d = din("ident", [64, 64])
    identf_d = din("identf", [128, 128], F32)
    subs_d = din("subs", [R, n_steps], F32)

    pose_d = nc.dram_tensor(
        "pose", [BL, n_steps * 8], F32, kind="ExternalOutput"
    ).ap()
    hlast_d = nc.dram_tensor("hlast", [R, F], F32, kind="ExternalOutput").ap()

    with tile.TileContext(nc) as tc:
        _body(nc, tc, n_steps, locals())
    nc.compile()
    return nc


def _body(nc, tc, n_steps, d):
    from contextlib import ExitStack

    ctx = ExitStack()
    with ctx:
        wp = ctx.enter_context(tc.tile_pool(name="weights", bufs=1))
        st = ctx.enter_context(tc.tile_pool(name="state", bufs=1))
        dram = ctx.enter_context(tc.tile_pool(name="dram", bufs=1, space="DRAM"))

        # ---- persistent weights ----
        def wload(ap_d, kt, n):
            t = wp.tile([128, kt, n], BF, tag=ap_d.tensor.name)
            for k in range(kt):
                nc.sync.dma_start(t[:, k, :], ap_d[k * 128 : (k + 1) * 128, :])
            return t

        w0 = wload(d["w0_d"], FK, H)
        w1 = wload(d["w1_d"], HK, H)
        w2 = wload(d["w2_d"], HK, F)
        wh1 = wload(d["wh1_d"], FK, G)
        wx2 = wload(d["wx2_d"], FK, G)
        wh2 = wload(d["wh2_d"], FK, G)
        rw1 = wload(d["rw1_d"], FK, 128)
        rw2 = wp.tile([128, 8], BF)
        nc.sync.dma_start(rw2[:, :], d["rw2_d"][:, :])

        def brow(ap_d, n):
            t = wp.tile([1, n], BF, tag=ap_d.tensor.name)
            nc.sync.dma_start(t[:, :], ap_d[:, :])
            return t

        b0r = brow(d["b0_d"], H)
        b1r = brow(d["b1_d"], H)
        b2r = brow(d["b2_d"], F)
        bs2r = brow(d["bs2_d"], G)
        bhn1r = brow(d["bhn1_d"], F)
        bhn2r = brow(d["bhn2_d"], F)
        rb1r = brow(d["rb1_d"], 128)
        rb2r = brow(d["rb2_d"], 8)

        ident = wp.tile([64, 64], BF)
        nc.sync.dma_start(ident[:, :], d["ident_d"][:, :])
        identf = wp.tile([128, 128], F32)
        nc.sync.dma_start(identf[:, :], d["identf_d"][:, :])
        ones = wp.tile([1, 128], BF)
        nc.vector.memset(ones[:, :], 1.0)
        subs = st.tile([R, n_steps], F32)
        nc.sync.dma_start(subs[:, :], d["subs_d"][:, :])

        # ---- state ----
        pose_sb = st.tile([R, n_steps * 8], F32)
        gx1_dram = dram.tile([n_steps * BL, G], BF)

        # ---- precompute GX1 = x @ Wx1.T + bs1 for all steps ----
        with (
            tc.tile_pool(name="pre", bufs=1) as pre,
            tc.tile_pool(name="pre2", bufs=2) as pre2,
            tc.tile_pool(name="pre_ps", bufs=2, space="PSUM") as pre_ps,
        ):
            nrows = n_steps * BL
            xT = pre.tile([128, FK, nrows], BF)
            for k in range(FK):
                nc.sync.dma_start(xT[:, k, :], d["xT_d"][k * 128 : (k + 1) * 128, :])
            wx1 = pre.tile([128, FK, G], BF)
            for k in range(FK):
                nc.sync.dma_start(
                    wx1[:, k, :], d["wx1_d"][k * 128 : (k + 1) * 128, :]
                )
            bs1r = pre.tile([1, G], BF)
            nc.sync.dma_start(bs1r[:, :], d["bs1_d"][:, :])
            for m in range(0, nrows, 128):
                mw = min(128, nrows - m)
                stage = pre2.tile([128, G], BF, tag="stage")
                for (c, wdt) in _nchunks(G):
                    ps = pre_ps.tile([128, 512], F32, tag="ps")
                    for k in range(FK):
                        nc.tensor.matmul(
                            ps[:mw, :wdt],
                            xT[:, k, m : m + mw],
                            wx1[:, k, c : c + wdt],
                            start=(k == 0),
                            stop=False,
                        )
                    nc.tensor.matmul(
                        ps[:mw, :wdt],
                        ones[:, :mw],
                        bs1r[:, c : c + wdt],
                        start=False,
                        stop=True,
                    )
                    nc.scalar.copy(stage[:mw, c : c + wdt], ps[:mw, :wdt])
                nc.sync.dma_start(gx1_dram[m : m + mw, :], stage[:mw, :])

        # ---- the scan ----
        # State lives transposed in a persistent PSUM bank: yT_psum
        # [128, FK, R] f32. Euler updates accumulate via fp32 transposes of
        # sub*f (start=False); the GRU rewrites it (first transpose
        # start=True clears the bank).
        ctx2 = ExitStack()
        with ctx2:
            ap = ctx2.enter_context(tc.tile_pool(name="act", bufs=2))
            sp = ctx2.enter_context(tc.tile_pool(name="small", bufs=2))
            gp = ctx2.enter_context(tc.tile_pool(name="gh", bufs=1))
            psp = ctx2.enter_context(tc.tile_pool(name="ps", bufs=3, space="PSUM"))
            trp = ctx2.enter_context(tc.tile_pool(name="tr", bufs=2, space="PSUM"))
            pst1 = ctx2.enter_context(tc.tile_pool(name="yps", bufs=1, space="PSUM"))

            ytp = pst1.tile([128, FK, R], F32)  # persistent transposed state
            ybm = pst1.tile([R, F], F32)  # batch-major h for GRU gates

            def cast_yT(sl=None, tag="yT"):
                # yT_psum -> sbuf bf16 stationaries (per-k so mms start early)
                if sl is None:
                    dst = ap.tile([128, FK, R], BF, tag=tag)
                    for k in range(FK):
                        nc.vector.tensor_copy(dst[:, k, :], ytp[:, k, :])
                else:
                    dst = ap.tile([128, FK, BL], BF, tag=tag)
                    for k in range(FK):
                        nc.vector.tensor_copy(dst[:, k, :], ytp[:, k, sl])
                return dst

            def mlp_layer(xT_t, kt, wT, brow_t, nout, outf, act, rows=R, p0=0):
                # outf[p0:p0+rows, nout] <- act(xT_t.T @ wT + brow)
                tp = (0, p0) if p0 else None
                for (c, wdt) in _nchunks(nout):
                    ps = psp.tile([64, 512], F32, tag="mm")
                    for k in range(kt):
                        nc.tensor.matmul(
                            ps[p0 : p0 + rows, :wdt],
                            xT_t[:, k, :rows],
                            wT[:, k, c : c + wdt],
                            start=(k == 0),
                            stop=False,
                            tile_position=tp,
                        )
                    nc.tensor.matmul(
                        ps[p0 : p0 + rows, :wdt],
                        ones[:, :rows],
                        brow_t[:, c : c + wdt],
                        start=False,
                        stop=True,
                        tile_position=tp,
                    )
                    nc.scalar.activation(
                        outf[p0 : p0 + rows, c : c + wdt],
                        ps[p0 : p0 + rows, :wdt],
                        act,
                    )

            def transpose_h(src_bf, kt, tag):
                # full-width [R, kt*128] bf16 -> [128, kt, R] bf16 stationaries
                pst = trp.tile([128, kt * R], BF, tag="tr")
                for k in range(kt):
                    nc.tensor.transpose(
                        pst[:, k * R : (k + 1) * R],
                        src_bf[:R, k * 128 : (k + 1) * 128],
                        ident[:R, :R],
                    )
                dst = ap.tile([128, kt, R], BF, tag=tag)
                h = kt // 2
                for i in range(2):
                    nc.vector.tensor_copy(
                        dst[:, i * h : (i + 1) * h, :],
                        pst[:, i * h * R : (i + 1) * h * R].rearrange(
                            "p (k r) -> p k r", k=h
                        ),
                    )
                return dst

            def gh_compute(p0, hT, wh, bhn, tag):
                # gh = h @ Wh.T (+bh_n on n-cols) -> sbuf bf16 [band, G]
                tp = (0, p0) if p0 else None
                pe = p0 + BL
                gh = gp.tile([R, G], BF, tag=tag)
                for ci, (c, wdt) in enumerate(_nchunks(G)):
                    ps = psp.tile([64, 512], F32, tag="mm")
                    for k in range(FK):
                        nc.tensor.matmul(
                            ps[p0:pe, :wdt],
                            hT[:, k, :],
                            wh[:, k, c : c + wdt],
                            start=(k == 0),
                            stop=(k == FK - 1 and c + wdt <= 2 * F),
                            tile_position=tp,
                        )
                    if c + wdt > 2 * F:  # n-section: add bh_n
                        nc.tensor.matmul(
                            ps[p0:pe, :wdt],
                            ones[:, :BL],
                            bhn[:, c - 2 * F : c - 2 * F + wdt],
                            start=False,
                            stop=True,
                            tile_position=tp,
                        )
                    nc.vector.tensor_copy(
                        gh[p0:pe, c : c + wdt], ps[p0:pe, :wdt]
                    )
                return gh

            def gates(p0, gx_sb, gh, hnew):
                pe = p0 + BL
                rz = sp.tile([R, 2 * F], BF, tag="rz")
                nc.vector.tensor_add(
                    rz[p0:pe, :], gx_sb[:, : 2 * F], gh[p0:pe, : 2 * F]
                )
                nc.scalar.activation(rz[p0:pe, :], rz[p0:pe, :], AF.Sigmoid)
                # n = tanh(gx_n + r*ghn)
                npre = sp.tile([R, F], BF, tag="npre")
                nc.vector.tensor_mul(
                    npre[p0:pe, :], rz[p0:pe, :F], gh[p0:pe, 2 * F :]
                )
                nc.vector.tensor_add(npre[p0:pe, :], npre[p0:pe, :], gx_sb[:, 2 * F :])
                nc.scalar.activation(npre[p0:pe, :], npre[p0:pe, :], AF.Tanh)
                # h' = n + z*(h - n)
                hmn = sp.tile([R, F], BF, tag="hmn")
                nc.vector.tensor_sub(hmn[p0:pe, :], ybm[p0:pe, :], npre[p0:pe, :])
                nc.vector.tensor_mul(hmn[p0:pe, :], rz[p0:pe, F : 2 * F], hmn[p0:pe, :])
                nc.vector.tensor_add(hnew[p0:pe, :], npre[p0:pe, :], hmn[p0:pe, :])

            for s in range(n_steps):
                gx1_sb = ap.tile([BL, G], BF, tag="gx1")
                nc.sync.dma_start(
                    gx1_sb[:, :], gx1_dram[s * BL : (s + 1) * BL, :]
                )
                # ---- KE Euler substeps ----
                for e in range(KE):
                    if s == 0 and e == 0:
                        yT = ap.tile([128, FK, R], BF, tag="yT")
                        nc.vector.memset(yT[:, :, :], 0.0)
                    else:
                        yT = cast_yT()
                    h1 = ap.tile([R, H], BF, tag="h1")
                    mlp_layer(yT, FK, w0, b0r, H, h1, AF.Tanh)
                    h1T = transpose_h(h1, HK, "h1T")
                    h2 = ap.tile([R, H], BF, tag="h2")
                    mlp_layer(h1T, HK, w1, b1r, H, h2, AF.Tanh)
                    h2T = transpose_h(h2, HK, "h2T")
                    # f = h2 @ W2.T + b2;  yT_psum += transpose(sub*f)
                    ftmp = sp.tile([R, F], F32, tag="ftmp")
                    for (c, wdt) in _nchunks(F):
                        ps = psp.tile([64, 512], F32, tag="mm")
                        for k in range(HK):
                            nc.tensor.matmul(
                                ps[:, :wdt],
                                h2T[:, k, :],
                                w2[:, k, c : c + wdt],
                                start=(k == 0),
                                stop=False,
                            )
                        nc.tensor.matmul(
                            ps[:, :wdt],
                            ones[:, :R],
                            b2r[:, c : c + wdt],
                            start=False,
                            stop=True,
                        )
                        nc.scalar.activation(
                            ftmp[:, c : c + wdt],
                            ps[:, :wdt],
                            AF.Copy,
                            scale=subs[:, s : s + 1],
                        )
                    for k in range(FK):
                        nc.tensor.matmul(
                            ytp[:, k, :],
                            ftmp[:, k * 128 : (k + 1) * 128],
                            identf[:R, :R],
                            is_transpose=True,
                            start=(s == 0 and e == 0 and k == 0),
                            stop=True,
                        )

                # ---- GRU ----
                yT = cast_yT()
                # batch-major h for the gates (fp32, via sbuf roundtrip)
                yTf = gp.tile([128, FK, R], F32, tag="yTf")
                nc.scalar.copy(yTf[:, :, :], ytp[:, :, :])
                for k in range(FK):
                    nc.tensor.transpose(
                        ybm[:, k * 128 : (k + 1) * 128],
                        yTf[:, k, :],
                        identf[:, :],
                    )
                hnew = ap.tile([R, F], F32, tag="hnew")
                gh1 = gh_compute(0, yT[:, :, :BL], wh1, bhn1r, "gh1")
                gh2 = gh_compute(BL, yT[:, :, BL:], wh2, bhn2r, "gh2")
                gates(0, gx1_sb[:, :], gh1, hnew)
                # rewrite yT_psum with layer-1 h' (clears the bank first)
                for k in range(FK):
                    nc.tensor.matmul(
                        ytp[:, k, :BL],
                        hnew[:BL, k * 128 : (k + 1) * 128],
                        identf[:BL, :BL],
                        is_transpose=True,
                        start=(k == 0),
                        stop=True,
                    )
                h1oT = cast_yT(sl=slice(0, BL), tag="h1oT")
                gx2_sb = ap.tile([R, G], BF, tag="gx2")
                mlp_layer(h1oT, FK, wx2, bs2r, G, gx2_sb, AF.Copy, rows=BL, p0=BL)
                gates(BL, gx2_sb[BL:, :], gh2, hnew)
                for k in range(FK):
                    nc.tensor.matmul(
                        ytp[:, k, BL:],
                        hnew[BL:, k * 128 : (k + 1) * 128],
                        identf[BL:R, BL:R],
                        is_transpose=True,
                        start=False,
                        stop=True,
                    )

                # ---- regressor on h2out (band 32:64) ----
                h2oT = cast_yT(sl=slice(BL, R), tag="h1oT")
                psr = psp.tile([64, 128], F32, tag="mm")
                for k in range(FK):
                    nc.tensor.matmul(
                        psr[BL:, :],
                        h2oT[:, k, :],
                        rw1[:, k, :],
                        start=(k == 0),
                        stop=False,
                        tile_position=(0, BL),
                    )
                nc.tensor.matmul(
                    psr[BL:, :],
                    ones[:, :BL],
                    rb1r[:, :],
                    start=False,
                    stop=True,
                    tile_position=(0, BL),
                )
                hid_bf = sp.tile([R, 128], BF, tag="hid")
                hid01 = sp.tile([R, 128], F32, tag="hid01")
                nc.vector.tensor_scalar_mul(hid01[BL:, :], psr[BL:, :], 0.1)
                nc.vector.tensor_max(hid_bf[BL:, :], psr[BL:, :], hid01[BL:, :])
                pstr = trp.tile([128, 32], BF, tag="tr")
                nc.tensor.transpose(
                    pstr[:, :], hid_bf[BL:, :], ident[BL:, BL:]
                )
                hidT = sp.tile([128, 32], BF, tag="hidT")
                nc.scalar.copy(hidT[:, :], pstr[:, :])
                psr2 = psp.tile([64, 8], F32, tag="mm")
                nc.tensor.matmul(
                    psr2[BL:, :], hidT[:, :], rw2[:, :],
                    start=True, stop=False, tile_position=(0, BL),
                )
                nc.tensor.matmul(
                    psr2[BL:, :], ones[:, :BL], rb2r[:, :],
                    start=False, stop=True, tile_position=(0, BL),
                )
                nc.vector.tensor_copy(
                    pose_sb[BL:, s * 8 : (s + 1) * 8], psr2[BL:, :]
                )
                if s == n_steps - 1:
                    nc.sync.dma_start(d["hlast_d"][:, :], hnew[:, :])

            nc.sync.dma_start(d["pose_d"][:, :], pose_sb[BL:, :])

# ---------------- host side ----------------


def _prep_core_inputs(inputs, core, n_steps=S):
    bf16 = ml_dtypes.bfloat16
    sl = slice(core * BL, (core + 1) * BL)
    fv, fi, ts = inputs["fv"][sl], inputs["fi"][sl], inputs["ts"][sl]
    x = np.concatenate([fv, fi], -1).astype(np.float32)  # [BL, S, F]
    x = np.swapaxes(x, 0, 1)[:n_steps]  # [S, BL, F]
    xT = np.ascontiguousarray(x.reshape(n_steps * BL, F).T).astype(bf16)
    dts = np.concatenate(
        [ts[:, 1:] - ts[:, :-1], np.zeros((BL, 1), np.float32)], 1
    )
    sub = (dts / KE).astype(np.float32)[:, :n_steps]  # [BL, n_steps]
    subs = np.concatenate([sub, sub], 0)  # [R, n_steps]

    bx, bh = inputs["gru_bx"], inputs["gru_bh"]

    def bsum(l):
        return np.concatenate(
            [bx[l][: 2 * F] + bh[l][: 2 * F], bx[l][2 * F :]]
        ).astype(np.float32)

    t = lambda a: np.ascontiguousarray(np.asarray(a, np.float32).T).astype(bf16)
    row = lambda v: np.asarray(v, np.float32).reshape(1, -1).astype(bf16)
    rw2 = np.zeros((128, 8), np.float32)
    rw2[:, :6] = inputs["reg_W2"].T
    rb2 = np.zeros((1, 8), np.float32)
    rb2[0, :6] = inputs["reg_b2"]
    return {
        "xT": xT,
        "w0T": t(inputs["ode_W0"]),
        "w1T": t(inputs["ode_W1"]),
        "w2T": t(inputs["ode_W2"]),
        "wx1T": t(inputs["gru_Wx"][0]),
        "wh1T": t(inputs["gru_Wh"][0]),
        "wx2T": t(inputs["gru_Wx"][1]),
        "wh2T": t(inputs["gru_Wh"][1]),
        "rw1T": t(inputs["reg_W1"]),
        "rw2T": rw2.astype(bf16),
        "b0r": row(inputs["ode_b0"]),
        "b1r": row(inputs["ode_b1"]),
        "b2r": row(inputs["ode_b2"]),
        "bs1r": row(bsum(0)),
        "bs2r": row(bsum(1)),
        "bhn1r": row(bh[0][2 * F :]),
        "bhn2r": row(bh[1][2 * F :]),
        "rb1r": row(inputs["reg_b1"]),
        "rb2r": rb2.astype(bf16),
        "ident": np.eye(64, dtype=bf16),
        "identf": np.eye(128, dtype=np.float32),
        "subs": subs,
    }


def run(inputs, n_steps=S, trace=False, nc=None, repeats=1, timings=None):
    inputs = {k: np.asarray(v) for k, v in inputs.items()}
    if nc is None:
        nc = build(n_steps)
    in_maps = [_prep_core_inputs(inputs, c, n_steps) for c in range(NCORES)]
    res = run_bass_kernel_spmd(
        nc, in_maps, core_ids=list(range(NCORES)), trace=trace
    )
    if repeats > 1 and timings is not None:
        import time as _time
        for _ in range(repeats - 1):
            t0 = _time.time()
            run_bass_kernel_spmd(
                nc, in_maps, core_ids=list(range(NCORES)), trace=False
            )
            timings.append(_time.time() - t0)
    pose = np.zeros((B, n_steps, 6), np.float32)
    hlast = np.zeros((L, B, F), np.float32)
    for c in range(NCORES):
        sl = slice(c * BL, (c + 1) * BL)
        pc = res.results[c]["pose"].reshape(BL, n_steps, 8)
        pose[sl] = pc[:, :, :6]
        hlast[:, sl] = res.results[c]["hlast"].reshape(L, BL, F)
    return (pose, hlast), res


def kernel(**inputs):
    (pose, hlast), _ = run(inputs)
    return pose, hlast
